# revision 15
# baseline (speedup 1.0000x reference)
"""GPT-2 attention block on 8 TRN2 NeuronCores.

Sharding (Megatron-style): core i owns heads (2i, 2i+1) for both batches.

Single fused pipeline over 512-token chunks n=0..7 (b-major):
 - q^T/k^T projection transposed (Wshard^T @ X^T); V projected directly to
   NATURAL layout (xT tile stationary, Wv moving) into [tok, 64|1] slots of
   v_aug, so no PE transposes are needed anywhere.
 - scores per (batch, head) transposed S^T[k, q], causal tiles only; the
   causal mask of each diagonal tile is ADDED ON PE (ident x maskT matmul
   accumulated into the score psum) instead of a DVE pass; one fused exp on
   ScalarE per k-tile (1/sqrt(64) folded into the activation scale).
 - AV in natural orientation: exp tile [k,q] stationary, [v|1] moving ->
   av psum [q, d|den]: half the matmul rows of the transposed form, and the
   softmax denominator lands per-token-per-partition for free; DVE does
   reciprocal + 2 scaled copies into av_sb (bf16, natural).
 - AV(n-1) is emitted AFTER scores(n) so exp(n-1) hides behind QKV+scores
   PE work (software pipeline, et tiles double-buffered across chunks).
 - AllToAll reshards to sequence parallelism; the receive side uses a
   single DMA-TRANSPOSE (XBAR) from the DRAM bounce straight into a^T in
   SBUF (no staging DMA, no PE transposes, no DVE copies).
 - output projection per 128-token tile; psum->sbuf copies on ScalarE
   (idle in the tail) and out DMAs on the Activation HWDGE queue.
Output per core j: [512, 1024] fp32 - rows 0:256 = batch0 tokens 256j..,
rows 256:384 = batch1 tokens 128j.., rows 384:512 = batch1 tokens
1024+128j..; host reassembles. Matmuls in bf16 (fp32 PSUM accumulation);
softmax in fp32. Post passes: ldweights dedup + splitting multi-wait
instructions into single-wait NoOps (this walrus build caps HW waits at 1).
"""

import numpy as np
import ml_dtypes

import concourse.bass as bass
import concourse.mybir as mybir
import concourse.tile as tile
from concourse.bass_utils import run_bass_kernel_spmd

BF16 = mybir.dt.bfloat16
F32 = mybir.dt.float32
AF = mybir.ActivationFunctionType

B, S, D, H = 2, 2048, 1024, 16
NT = B * S          # 4096 tokens, b-major
NCORES = 8
DK = D // H         # 64
NEG = -1.0e30
SCALE = 0.125       # 1/sqrt(64)

_CACHE = {}
_NO_COLLECTIVE = False


def _build(debug_dumps=False):
    nc = bass.Bass("TRN2", target_bir_lowering=False, debug=False,
                   num_devices=NCORES)

    xT = nc.dram_tensor("xT", [D, NT], BF16, kind="ExternalInput").ap()
    # wqkv pre-swizzled host-side to [128(p), 3(m), 8(kt), 128(col)]
    wqkv = nc.dram_tensor("wqkv", [128, 3072], BF16, kind="ExternalInput").ap()
    wp = nc.dram_tensor("wp", [D, D], BF16, kind="ExternalInput").ap()
    # cbf16: identity[128] | maskT[128] (bf16)
    cbf16 = nc.dram_tensor("cbf16", [128, 256], BF16, kind="ExternalInput").ap()
    # cf32: bqkv[3] | bv broadcast [128]
    cf32 = nc.dram_tensor("cf32", [128, 131], F32, kind="ExternalInput").ap()
    out = nc.dram_tensor("out", [512, 1024], F32, kind="ExternalOutput").ap()
    dbg = None
    if debug_dumps:
        dbg = {
            "dbg_qkT": nc.dram_tensor(
                "dbg_qkT", [128, 2, NT], BF16, kind="ExternalOutput").ap(),
            "dbg_vaug": nc.dram_tensor(
                "dbg_vaug", [128, 32, 130], BF16, kind="ExternalOutput").ap(),
            "dbg_av": nc.dram_tensor(
                "dbg_av", [128, 32, 128], BF16, kind="ExternalOutput").ap(),
            "dbg_aT": nc.dram_tensor(
                "dbg_aT", [128, 8, 512], BF16, kind="ExternalOutput").ap(),
        }

    with tile.TileContext(nc) as tc:
        _body(tc, out, xT, wqkv, wp, cbf16, cf32, dbg)
    _dedup_ldweights(nc)
    _split_multi_waits(nc)
    return nc


def _dedup_ldweights(nc):
    """Drop a back-to-back identical, wait-free Ldweights (weights already
    resident; only Matmults in between; transposes clobber -> reset)."""
    for f in nc.m.functions:
        for bb in f.blocks:
            insts = bb.instructions
            new = []
            changed = False
            last_w = None
            for inst in insts:
                nm = inst.__class__.__name__
                if getattr(inst, "engine", None) == mybir.EngineType.PE:
                    if nm == "InstLdweights":
                        si = inst.sync_info
                        key = repr(inst.ins)
                        no_waits = si is None or not si.on_wait
                        no_upd = si is None or not si.on_update
                        if key == last_w and no_waits and no_upd:
                            changed = True
                            continue  # drop duplicate load
                        last_w = key
                    elif nm == "InstMatmult":
                        if getattr(inst, "is_transpose", False):
                            last_w = None
                    else:
                        last_w = None
                new.append(inst)
            if changed:
                bb.instructions = new


def _split_multi_waits(nc):
    """Walrus caps HW sync waits at 1 per instruction: hoist extras onto
    dedicated NoOps inserted just before the offender (same engine queue)."""
    import bass_rust
    nid = [0]
    for f in nc.m.functions:
        for bb in f.blocks:
            insts = bb.instructions
            new = []
            changed = False
            for inst in insts:
                si = getattr(inst, "sync_info", None)
                if si is not None and len(si.on_wait) > 1:
                    changed = True
                    waits = list(si.on_wait)
                    for w in waits[:-1]:
                        nid[0] += 1
                        nop = mybir.InstNoOp(
                            name=f"I-waitnop-{nid[0]}", ins=[], outs=[])
                        nop.engine = inst.engine
                        nop.sync_info = bass_rust.SyncInfo(
                            on_wait=[w], on_update=[])
                        new.append(nop)
                    inst.sync_info = bass_rust.SyncInfo(
                        on_wait=[waits[-1]], on_update=list(si.on_update))
                new.append(inst)
            if changed:
                bb.instructions = new


def _body(tc, out, xT, wqkv, wp, cbf16, cf32, dbg=None):
    nc = tc.nc

    with (
        tc.tile_pool(name="persist", bufs=1) as persist,
        tc.tile_pool(name="expp", bufs=30) as expp_pool,
        tc.tile_pool(name="smalls", bufs=8) as smalls_pool,
        tc.tile_pool(name="outs", bufs=3) as outs_pool,
        tc.tile_pool(name="pss", bufs=2, space="PSUM") as pss_pool,
        tc.tile_pool(name="qkvp", bufs=2, space="PSUM") as qkvp_pool,
        tc.tile_pool(name="psv", bufs=2, space="PSUM") as psv_pool,
        tc.tile_pool(name="dram", bufs=1, space="DRAM") as dram_pool,
    ):
        # ---- persistent SBUF ----
        xT_sb = persist.tile([128, 8, NT], BF16)        # X^T, D-tile major
        wqkv_sb = persist.tile([128, 3, 8, 128], BF16)  # [p, m, kt, col]
        wp_sb = persist.tile([128, 8, 1024], BF16)
        qkT_sb = persist.tile([128, 2, NT], BF16)       # q^T | k^T rows
        v_aug = persist.tile([128, 32, 2, 65], BF16)    # [tok, h, v|1]
        av_sb = persist.tile([128, 32, 128], BF16)      # av natural
        # a^T after a2a: contiguous per a2a stage (XBAR needs dense dst).
        # batch 1 reshards in four 512-token pieces, slot order c3,c2,c1,c0.
        aT0 = persist.tile([128, 8, 256], BF16)
        aT1 = persist.tile([128, 4, 8, 64], BF16)     # XBAR landing, per slot
        aT1m = persist.tile([128, 2, 8, 128], BF16)   # pair-merged for proj
        aT_of = {0: aT0[:, :, :], "q3": aT1[:, 0, :, :], "q2": aT1[:, 1, :, :],
                 "q1": aT1[:, 2, :, :], "q0": aT1[:, 3, :, :]}
        slot_of = {"q3": 0, "q2": 1, "q1": 2, "q0": 3}
        cbf16_sb = persist.tile([128, 256], BF16)
        cf32_sb = persist.tile([128, 131], F32)
        ident_sb = cbf16_sb[:, 0:128]
        maskT_sb = cbf16_sb[:, 128:256]
        bqkv_sb = cf32_sb[:, 0:3]
        bv_sb = cf32_sb[:, 3:131]                       # v-bias bcast [128,128]

        av_bounce = {0: dram_pool.tile([S, 128], BF16, name="avb0")}
        recv_bounce = {0: dram_pool.tile([S, 128], BF16, name="rcv0")}
        for c in range(4):
            av_bounce[f"q{c}"] = dram_pool.tile(
                [512, 128], BF16, name=f"avbq{c}")
            recv_bounce[f"q{c}"] = dram_pool.tile(
                [512, 128], BF16, name=f"rcvq{c}")

        # ones columns of v_aug (v slots overwritten per chunk)
        nc.vector.memset(v_aug[:, :, :, 64:65], 1.0)

        # ---- ACT warmup: attach table-load pseudos to wait-free instructions
        warm = smalls_pool.tile([1, 2], F32, tag="warm")
        nc.vector.memset(warm[:, 0:1], 0.0)
        nc.scalar.activation(warm[:, 1:2], warm[:, 0:1], AF.Identity)
        nc.scalar.activation(warm[:, 1:2], warm[:, 0:1], AF.Exp)
        nc.scalar.activation(warm[:, 1:2], warm[:, 0:1], AF.Copy)

        # ---- input DMAs; first QKV matmul needs wqkv m=0 + first xT piece.
        nc.sync.dma_start(
            wqkv_sb[:, 0, :, :],
            wqkv[:, 0:1024].rearrange("p (kt n) -> p kt n", kt=8))
        nc.sync.dma_start(
            xT_sb[:, :, 0:256],
            xT[:, 0:256].rearrange("(kt p) w -> p kt w", p=128))
        for m in (1, 2):
            nc.sync.dma_start(
                wqkv_sb[:, m, :, :],
                wqkv[:, m * 1024:(m + 1) * 1024]
                .rearrange("p (kt n) -> p kt n", kt=8))
        nc.sync.dma_start(
            xT_sb[:, :, 256:512],
            xT[:, 256:512].rearrange("(kt p) w -> p kt w", p=128))
        nc.sync.dma_start(cbf16_sb[:, :], cbf16[:, :])
        nc.sync.dma_start(cf32_sb[:, :], cf32[:, :])
        for n in range(1, 8):
            nc.sync.dma_start(
                xT_sb[:, :, n * 512:(n + 1) * 512],
                xT[:, n * 512:(n + 1) * 512]
                .rearrange("(kt p) w -> p kt w", p=128))
        nc.sync.dma_start(wp_sb[:, :, :],
                          wp.rearrange("(kt p) n -> p kt n", p=128))

        def qkv_qk(n):
            # q^T/k^T chunk n (tokens 512n..): [128, 512] per m
            for m in range(2):
                ps = qkvp_pool.tile([128, 512], F32, tag="qkvp")
                pieces = ((0, 256), (256, 512)) if n == 0 else ((0, 512),)
                for lo, hi in pieces:
                    for kt in range(8):
                        nc.tensor.matmul(
                            ps[:, lo:hi],
                            wqkv_sb[:, m, kt, :],
                            xT_sb[:, kt, n * 512 + lo:n * 512 + hi],
                            start=(kt == 0), stop=(kt == 7),
                        )
                nc.vector.tensor_scalar_add(
                    qkT_sb[:, m, n * 512:(n + 1) * 512],
                    ps[:, :], bqkv_sb[:, m:m + 1])

        def vnat(n):
            # V natural per 128-token tile: xT tile stationary, Wv moving
            for t in range(4 * n, 4 * n + 4):
                ps_v = psv_pool.tile([128, 128], F32, tag="psv")
                for kt in range(8):
                    nc.tensor.matmul(
                        ps_v[:, :],
                        xT_sb[:, kt, t * 128:(t + 1) * 128],
                        wqkv_sb[:, 2, kt, :],
                        start=(kt == 0), stop=(kt == 7),
                    )
                # psum + v-bias(bcast) -> v_aug [tok, h, 0:64] (one 3D AP)
                nc.vector.tensor_tensor(
                    v_aug[:, t, :, 0:64],
                    ps_v[:, :].rearrange("p (h d) -> p h d", h=2),
                    bv_sb.rearrange("p (h d) -> p h d", h=2),
                    mybir.AluOpType.add)

        def scores(b, c, ets):
            # S^T[k, q] for chunk c of batch b; exp tiles into `ets`
            tok0 = b * S
            nk = 4 * c + 4
            q0 = tok0 + c * 512
            for ki in range(nk):
                off = max(0, (ki - 4 * c)) * 128
                w = 512 - off
                ps_s = pss_pool.tile([128, 2, 512], F32, tag="pss")
                for h in range(2):
                    hp = h * 64
                    kslice = qkT_sb[hp:hp + 64, 1,
                                    tok0 + ki * 128: tok0 + (ki + 1) * 128]
                    if ki >= 4 * c:
                        # diagonal tile: scores on first 128 cols, then the
                        # causal mask accumulated on PE, then the rest.
                        nc.tensor.matmul(
                            ps_s[:, h, 0:128], kslice,
                            qkT_sb[hp:hp + 64, 0, q0 + off: q0 + off + 128],
                            start=True, stop=False)
                        nc.tensor.matmul(
                            ps_s[:, h, 0:128], ident_sb, maskT_sb,
                            start=False, stop=True)
                        if w > 128:
                            nc.tensor.matmul(
                                ps_s[:, h, 128:w], kslice,
                                qkT_sb[hp:hp + 64, 0, q0 + off + 128: q0 + 512],
                                start=True, stop=True)
                    else:
                        nc.tensor.matmul(
                            ps_s[:, h, :w], kslice,
                            qkT_sb[hp:hp + 64, 0, q0 + off: q0 + 512],
                            start=True, stop=True)
                et = expp_pool.tile([128, 2, 512], BF16, tag="expp")
                nc.scalar.activation(
                    et[:, :, :w], ps_s[:, :, :w], AF.Exp, scale=SCALE)
                ets[ki] = (et, off)

        def av(b, c, ets):
            # natural AV per (qtile, head): et stationary, [v|1] moving
            for qq in range(4):
                nk = 4 * c + qq + 1
                ps_a = psv_pool.tile([128, 2, 65], F32, tag="psv")
                for h in range(2):
                    for ki in range(nk):
                        et, off = ets[ki]
                        nc.tensor.matmul(
                            ps_a[:, h, :],
                            et[:, h, qq * 128 - off: qq * 128 - off + 128],
                            v_aug[:, b * 16 + ki, h, :],
                            start=(ki == 0), stop=(ki == nk - 1),
                        )
                recip = smalls_pool.tile([128, 2], F32, tag="recip")
                nc.vector.reciprocal(recip[:, :], ps_a[:, :, 64])
                tindex = b * 16 + c * 4 + qq
                for h in range(2):
                    nc.vector.tensor_scalar_mul(
                        av_sb[:, tindex, h * 64:(h + 1) * 64],
                        ps_a[:, h, 0:64], recip[:, h:h + 1])

        def a2a(key, t0, nt):
            # all-to-all over av token-tiles [t0, t0+nt): nt*16-token blocks
            nc.sync.dma_start(
                av_bounce[key].rearrange("(t p) d -> p t d", p=128),
                av_sb[:, t0:t0 + nt, :])
            if _NO_COLLECTIVE:
                nc.sync.dma_start(recv_bounce[key][:, :], av_bounce[key][:, :])
            else:
                nc.gpsimd.collective_compute(
                    "AllToAll", mybir.AluOpType.bypass,
                    replica_groups=[list(range(NCORES))],
                    ins=[av_bounce[key][:, :].opt()],
                    outs=[recv_bounce[key][:, :].opt()],
                )

        def recv_stage(key):
            # DMA-transpose DRAM bounce [(s t), d] -> aT[:, s, t]
            nc.sync.dma_start(
                aT_of[key], recv_bounce[key][:, :], transpose=True)
            if key in slot_of:
                # merge 64-token slot into the pair-contiguous proj layout
                i = slot_of[key]
                nc.vector.tensor_copy(
                    aT1m[:, i // 2, :, (i % 2) * 64:(i % 2) * 64 + 64],
                    aT1[:, i, :, :])

        def proj(r0, a_slice):
            # out rows r0:r0+128 from a^T slices {s: [128, .., 128-token] AP}
            for n2 in range(2):
                ps = qkvp_pool.tile([128, 512], F32, tag="qkvp")
                for s in range(8):
                    nc.tensor.matmul(
                        ps[:, :],
                        a_slice(s),
                        wp_sb[:, s, n2 * 512:(n2 + 1) * 512],
                        start=(s == 0), stop=(s == 7),
                    )
                o_sb = outs_pool.tile([128, 512], F32, tag="outs")
                nc.scalar.activation(o_sb[:, :], ps[:, :], AF.Copy)
                nc.scalar.dma_start(
                    out[r0:r0 + 128, n2 * 512:(n2 + 1) * 512],
                    o_sb[:, :])

        # ---- fused pipeline. Batch 0: per-chunk QKV/scores with AV lagging
        # one chunk so exp hides behind PE work. Batch 1: QKV bunched, then
        # chunks in DESCENDING order (c3..c0) so the last a2a piece and its
        # exp are the smallest; the last two projections are emitted after
        # the final a2a so PE stays busy through the collective chain.
        ets = {}
        for n in range(4):
            qkv_qk(n)
            vnat(n)
            ets[n] = {}
            scores(0, n, ets[n])
            if n >= 1:
                av(0, n - 1, ets.pop(n - 1))
        qkv_qk(4); vnat(4)
        qkv_qk(5); vnat(5)
        av(0, 3, ets.pop(3))
        a2a(0, 0, 16)
        recv_stage(0)
        qkv_qk(6); vnat(6)
        ets[2] = {}
        scores(1, 2, ets[2])
        qkv_qk(7); vnat(7)
        ets[3] = {}
        scores(1, 3, ets[3])
        av(1, 2, ets.pop(2))
        a2a("q2", 24, 4)
        recv_stage("q2")
        ets[1] = {}
        scores(1, 1, ets[1])
        av(1, 3, ets.pop(3))
        a2a("q3", 28, 4)
        recv_stage("q3")
        proj(0, lambda s: aT0[:, s, 0:128])
        ets[0] = {}
        scores(1, 0, ets[0])
        av(1, 1, ets.pop(1))
        a2a("q1", 20, 4)
        recv_stage("q1")
        proj(128, lambda s: aT0[:, s, 128:256])
        av(1, 0, ets.pop(0))
        a2a("q0", 16, 4)
        recv_stage("q0")
        # rows 256:384 = [c3 | c2] tokens, rows 384:512 = [c1 | c0]
        proj(256, lambda s: aT1m[:, 0, s, :])
        proj(384, lambda s: aT1m[:, 1, s, :])

        if dbg is not None:
            nc.sync.dma_start(
                dbg["dbg_qkT"].rearrange("p m n -> p (m n)"),
                qkT_sb[:, :, :].rearrange("p m n -> p (m n)"))
            nc.sync.dma_start(
                dbg["dbg_vaug"].rearrange("p t d -> p (t d)"),
                v_aug[:, :, :, :].rearrange("p t h d -> p (t h d)"))
            nc.sync.dma_start(
                dbg["dbg_av"].rearrange("p t d -> p (t d)"),
                av_sb[:, :, :].rearrange("p t d -> p (t d)"))
            nc.sync.dma_start(
                dbg["dbg_aT"][:, :, 0:256].rearrange("p s n -> p (s n)"),
                aT0[:, :, :].rearrange("p s n -> p (s n)"))
            for i in range(4):
                nc.sync.dma_start(
                    dbg["dbg_aT"][:, :, 256 + 64 * i: 320 + 64 * i]
                    .rearrange("p s n -> p (s n)"),
                    aT1[:, i, :, :].rearrange("p s n -> p (s n)"))


def _prep_inputs(hidden_states, c_attn_w, c_attn_b, c_proj_w):
    bf16 = ml_dtypes.bfloat16
    x = np.asarray(hidden_states, dtype=np.float32).reshape(NT, D)
    xT = np.ascontiguousarray(x.T).astype(bf16)
    wp = np.ascontiguousarray(np.asarray(c_proj_w, dtype=np.float32)).astype(bf16)
    identity = np.eye(128, dtype=np.float32)
    # maskT[p, f]: S^T diagonal tile entry (k=p, q=f) masked iff q < k
    p = np.arange(128)
    maskT = np.where(p[None, :] >= p[:, None], 0.0, NEG).astype(np.float32)
    cbf16 = np.ascontiguousarray(
        np.concatenate([identity, maskT], axis=1)).astype(bf16)

    w = np.asarray(c_attn_w, dtype=np.float32)
    bb = np.asarray(c_attn_b, dtype=np.float32)
    in_maps = []
    for i in range(NCORES):
        cols = np.r_[i * 128:(i + 1) * 128]
        # pre-swizzled [p(contraction), m, kt, col] -> [128, 3*8*128]
        wshard = np.stack(
            [w[:, cols], w[:, D + cols], w[:, 2 * D + cols]], axis=0
        ).reshape(3, 8, 128, 128).transpose(2, 0, 1, 3).reshape(128, 3072)
        bshard = np.stack(
            [bb[cols], bb[D + cols], bb[2 * D + cols]], axis=1)  # [128, 3]
        bv_bcast = np.tile(bb[2 * D + cols][None, :], (128, 1))  # [128, 128]
        cf32 = np.ascontiguousarray(
            np.concatenate([bshard, bv_bcast], axis=1)).astype(np.float32)
        in_maps.append({
            "xT": xT,
            "wqkv": np.ascontiguousarray(wshard).astype(bf16),
            "wp": wp,
            "cbf16": cbf16,
            "cf32": cf32,
        })
    return in_maps


def kernel(hidden_states, c_attn_w, c_attn_b, c_proj_w, c_proj_b, _trace=False):
    if "nc" not in _CACHE:
        _CACHE["nc"] = _build()
    nc = _CACHE["nc"]
    in_maps = _prep_inputs(hidden_states, c_attn_w, c_attn_b, c_proj_w)
    try:
        res = run_bass_kernel_spmd(nc, in_maps, core_ids=list(range(NCORES)),
                                   trace=_trace)
    except (ImportError, ModuleNotFoundError):
        # NTFF profiling hook unavailable in this container
        res = run_bass_kernel_spmd(nc, in_maps, core_ids=list(range(NCORES)),
                                   trace=False)
    _CACHE["last_result"] = res
    # core j's output rows: [0:256] = batch0 tokens 256j..;
    # [256+64i : 320+64i] = batch1 tokens 512*(3-i) + 64j .. (i = 0..3)
    full = np.empty((NT, D), dtype=np.float32)
    for j in range(NCORES):
        o = res.results[j]["out"]
        full[256 * j:256 * (j + 1)] = o[0:256]
        for i in range(4):
            t0 = S + 512 * (3 - i) + 64 * j
            full[t0:t0 + 64] = o[256 + 64 * i: 320 + 64 * i]
    full = full + np.asarray(c_proj_b, dtype=np.float32)[None, :]
    return full.reshape(B, S, D).astype(np.float32)


# revision 21
# speedup vs baseline: 1.0274x; 1.0274x over previous
"""GPT-2 attention block on 8 TRN2 NeuronCores.

Sharding (Megatron-style): core i owns heads (2i, 2i+1) for both batches.

Single fused pipeline over 512-token chunks n=0..7 (b-major):
 - q^T/k^T projection transposed (Wshard^T @ X^T); V projected directly to
   NATURAL layout (xT tile stationary, Wv moving) into [tok, 64|1] slots of
   v_aug, so no PE transposes are needed anywhere.
 - scores per (batch, head) transposed S^T[k, q], causal tiles only; the
   causal mask of each diagonal tile is ADDED ON PE (ident x maskT matmul
   accumulated into the score psum) instead of a DVE pass; one fused exp on
   ScalarE per k-tile (1/sqrt(64) folded into the activation scale).
 - AV in natural orientation: exp tile [k,q] stationary, [v|1] moving ->
   av psum [q, d|den]: half the matmul rows of the transposed form, and the
   softmax denominator lands per-token-per-partition for free; DVE does
   reciprocal + 2 scaled copies into av_sb (bf16, natural).
 - AV(n-1) is emitted AFTER scores(n) so exp(n-1) hides behind QKV+scores
   PE work (software pipeline, et tiles double-buffered across chunks).
 - AllToAll reshards to sequence parallelism; the receive side uses a
   single DMA-TRANSPOSE (XBAR) from the DRAM bounce straight into a^T in
   SBUF (no staging DMA, no PE transposes, no DVE copies).
 - output projection per 128-token tile; psum->sbuf copies on ScalarE
   (idle in the tail) and out DMAs on the Activation HWDGE queue.
Output per core j: [512, 1024] fp32 - rows 0:256 = batch0 tokens 256j..,
rows 256:384 = batch1 tokens 128j.., rows 384:512 = batch1 tokens
1024+128j..; host reassembles. Matmuls in bf16 (fp32 PSUM accumulation);
softmax in fp32. Post passes: ldweights dedup + splitting multi-wait
instructions into single-wait NoOps (this walrus build caps HW waits at 1).
"""

import numpy as np
import ml_dtypes

import concourse.bass as bass
import concourse.mybir as mybir
import concourse.tile as tile
from concourse.bass_utils import run_bass_kernel_spmd

BF16 = mybir.dt.bfloat16
F32 = mybir.dt.float32
AF = mybir.ActivationFunctionType

B, S, D, H = 2, 2048, 1024, 16
NT = B * S          # 4096 tokens, b-major
NCORES = 8
DK = D // H         # 64
NEG = -1.0e30
SCALE = 0.125       # 1/sqrt(64)

_CACHE = {}
_NO_COLLECTIVE = False


def _build(debug_dumps=False):
    nc = bass.Bass("TRN2", target_bir_lowering=False, debug=False,
                   num_devices=NCORES)

    xT = nc.dram_tensor("xT", [D, NT], BF16, kind="ExternalInput").ap()
    # wqkv pre-swizzled host-side to [128(p), 3(m), 8(kt), 128(col)]
    wqkv = nc.dram_tensor("wqkv", [128, 3072], BF16, kind="ExternalInput").ap()
    wp = nc.dram_tensor("wp", [D, D], BF16, kind="ExternalInput").ap()
    # cbf16: identity[128] | maskT[128] (bf16)
    cbf16 = nc.dram_tensor("cbf16", [128, 256], BF16, kind="ExternalInput").ap()
    # cf32: bqkv[3] | bv broadcast [128]
    cf32 = nc.dram_tensor("cf32", [128, 131], F32, kind="ExternalInput").ap()
    out = nc.dram_tensor("out", [512, 1024], F32, kind="ExternalOutput").ap()
    dbg = None
    if debug_dumps:
        dbg = {
            "dbg_qkT": nc.dram_tensor(
                "dbg_qkT", [128, 2, NT], BF16, kind="ExternalOutput").ap(),
            "dbg_vaug": nc.dram_tensor(
                "dbg_vaug", [128, 32, 130], BF16, kind="ExternalOutput").ap(),
            "dbg_av": nc.dram_tensor(
                "dbg_av", [128, 32, 128], BF16, kind="ExternalOutput").ap(),
            "dbg_aT": nc.dram_tensor(
                "dbg_aT", [128, 8, 512], BF16, kind="ExternalOutput").ap(),
        }

    with tile.TileContext(nc) as tc:
        _body(tc, out, xT, wqkv, wp, cbf16, cf32, dbg)
    _dedup_ldweights(nc)
    _split_multi_waits(nc)
    return nc


def _dedup_ldweights(nc):
    """Drop a back-to-back identical, wait-free Ldweights (weights already
    resident; only Matmults in between; transposes clobber -> reset)."""
    for f in nc.m.functions:
        for bb in f.blocks:
            insts = bb.instructions
            new = []
            changed = False
            last_w = None
            for inst in insts:
                nm = inst.__class__.__name__
                if getattr(inst, "engine", None) == mybir.EngineType.PE:
                    if nm == "InstLdweights":
                        si = inst.sync_info
                        key = repr(inst.ins)
                        no_waits = si is None or not si.on_wait
                        no_upd = si is None or not si.on_update
                        if key == last_w and no_waits and no_upd:
                            changed = True
                            continue  # drop duplicate load
                        last_w = key
                    elif nm == "InstMatmult":
                        if getattr(inst, "is_transpose", False):
                            last_w = None
                    else:
                        last_w = None
                new.append(inst)
            if changed:
                bb.instructions = new


def _split_multi_waits(nc):
    """Walrus caps HW sync waits at 1 per instruction: hoist extras onto
    dedicated NoOps inserted just before the offender (same engine queue)."""
    import bass_rust
    nid = [0]
    for f in nc.m.functions:
        for bb in f.blocks:
            insts = bb.instructions
            new = []
            changed = False
            for inst in insts:
                si = getattr(inst, "sync_info", None)
                if si is not None and len(si.on_wait) > 1:
                    changed = True
                    waits = list(si.on_wait)
                    for w in waits[:-1]:
                        nid[0] += 1
                        nop = mybir.InstNoOp(
                            name=f"I-waitnop-{nid[0]}", ins=[], outs=[])
                        nop.engine = inst.engine
                        nop.sync_info = bass_rust.SyncInfo(
                            on_wait=[w], on_update=[])
                        new.append(nop)
                    inst.sync_info = bass_rust.SyncInfo(
                        on_wait=[waits[-1]], on_update=list(si.on_update))
                new.append(inst)
            if changed:
                bb.instructions = new


def _body(tc, out, xT, wqkv, wp, cbf16, cf32, dbg=None):
    nc = tc.nc

    with (
        tc.tile_pool(name="persist", bufs=1) as persist,
        tc.tile_pool(name="expp", bufs=30) as expp_pool,
        tc.tile_pool(name="smalls", bufs=8) as smalls_pool,
        tc.tile_pool(name="outs", bufs=3) as outs_pool,
        tc.tile_pool(name="pss", bufs=2, space="PSUM") as pss_pool,
        tc.tile_pool(name="qkvp", bufs=2, space="PSUM") as qkvp_pool,
        tc.tile_pool(name="psv", bufs=2, space="PSUM") as psv_pool,
        tc.tile_pool(name="dram", bufs=1, space="DRAM") as dram_pool,
    ):
        # ---- persistent SBUF ----
        xT_sb = persist.tile([128, 8, NT], BF16)        # X^T, D-tile major
        wqkv_sb = persist.tile([128, 3, 8, 128], BF16)  # [p, m, kt, col]
        wp_sb = persist.tile([128, 8, 1024], BF16)
        qkT_sb = persist.tile([128, 2, NT], BF16)       # q^T | k^T rows
        v_aug = persist.tile([128, 32, 2, 65], BF16)    # [tok, h, v|1]
        av_sb = persist.tile([128, 32, 128], BF16)      # av natural
        # a^T after a2a: contiguous per a2a stage (XBAR needs dense dst).
        # batch 1 reshards in four 512-token pieces, slot order c3,c2,c1,c0.
        aT0 = persist.tile([128, 8, 256], BF16)
        aT1a = persist.tile([128, 8, 128], BF16)      # b1 tokens 1024:2048
        aT1b = persist.tile([128, 8, 128], BF16)      # b1 tokens 0:1024
        aT_of = {0: aT0[:, :, :], "1a": aT1a[:, :, :], "1b": aT1b[:, :, :]}
        cbf16_sb = persist.tile([128, 256], BF16)
        cf32_sb = persist.tile([128, 131], F32)
        ident_sb = cbf16_sb[:, 0:128]
        maskT_sb = cbf16_sb[:, 128:256]
        bqkv_sb = cf32_sb[:, 0:3]
        bv_sb = cf32_sb[:, 3:131]                       # v-bias bcast [128,128]

        av_bounce = {0: dram_pool.tile([S, 128], BF16, name="avb0"),
                     "1a": dram_pool.tile([S // 2, 128], BF16, name="avb1a"),
                     "1b": dram_pool.tile([S // 2, 128], BF16, name="avb1b")}
        recv_bounce = {0: dram_pool.tile([S, 128], BF16, name="rcv0"),
                       "1a": dram_pool.tile([S // 2, 128], BF16, name="rcv1a"),
                       "1b": dram_pool.tile([S // 2, 128], BF16, name="rcv1b")}

        # ones columns of v_aug (v slots overwritten per chunk)
        nc.vector.memset(v_aug[:, :, :, 64:65], 1.0)

        # ---- ACT warmup: attach table-load pseudos to wait-free instructions
        warm = smalls_pool.tile([1, 2], F32, tag="warm")
        nc.vector.memset(warm[:, 0:1], 0.0)
        nc.scalar.activation(warm[:, 1:2], warm[:, 0:1], AF.Identity)
        nc.scalar.activation(warm[:, 1:2], warm[:, 0:1], AF.Exp)
        nc.scalar.activation(warm[:, 1:2], warm[:, 0:1], AF.Copy)

        # ---- input DMAs; first QKV matmul needs wqkv m=0 + first xT piece.
        nc.sync.dma_start(
            wqkv_sb[:, 0, :, :],
            wqkv[:, 0:1024].rearrange("p (kt n) -> p kt n", kt=8))
        nc.sync.dma_start(
            xT_sb[:, :, 0:256],
            xT[:, 0:256].rearrange("(kt p) w -> p kt w", p=128))
        for m in (1, 2):
            nc.sync.dma_start(
                wqkv_sb[:, m, :, :],
                wqkv[:, m * 1024:(m + 1) * 1024]
                .rearrange("p (kt n) -> p kt n", kt=8))
        nc.sync.dma_start(
            xT_sb[:, :, 256:512],
            xT[:, 256:512].rearrange("(kt p) w -> p kt w", p=128))
        nc.sync.dma_start(cbf16_sb[:, :], cbf16[:, :])
        nc.sync.dma_start(cf32_sb[:, :], cf32[:, :])
        for n in range(1, 8):
            nc.sync.dma_start(
                xT_sb[:, :, n * 512:(n + 1) * 512],
                xT[:, n * 512:(n + 1) * 512]
                .rearrange("(kt p) w -> p kt w", p=128))
        nc.sync.dma_start(wp_sb[:, :, :],
                          wp.rearrange("(kt p) n -> p kt n", p=128))

        def qkv_qk(n):
            # q^T/k^T chunk n (tokens 512n..): [128, 512] per m
            for m in range(2):
                ps = qkvp_pool.tile([128, 512], F32, tag="qkvp")
                pieces = ((0, 256), (256, 512)) if n == 0 else ((0, 512),)
                for lo, hi in pieces:
                    for kt in range(8):
                        nc.tensor.matmul(
                            ps[:, lo:hi],
                            wqkv_sb[:, m, kt, :],
                            xT_sb[:, kt, n * 512 + lo:n * 512 + hi],
                            start=(kt == 0), stop=(kt == 7),
                        )
                nc.vector.tensor_scalar_add(
                    qkT_sb[:, m, n * 512:(n + 1) * 512],
                    ps[:, :], bqkv_sb[:, m:m + 1])

        def vnat(n):
            # V natural per 128-token tile: xT tile stationary, Wv moving
            for t in range(4 * n, 4 * n + 4):
                ps_v = psv_pool.tile([128, 128], F32, tag="psv")
                for kt in range(8):
                    nc.tensor.matmul(
                        ps_v[:, :],
                        xT_sb[:, kt, t * 128:(t + 1) * 128],
                        wqkv_sb[:, 2, kt, :],
                        start=(kt == 0), stop=(kt == 7),
                    )
                # psum + v-bias(bcast) -> v_aug [tok, h, 0:64] (one 3D AP)
                nc.vector.tensor_tensor(
                    v_aug[:, t, :, 0:64],
                    ps_v[:, :].rearrange("p (h d) -> p h d", h=2),
                    bv_sb.rearrange("p (h d) -> p h d", h=2),
                    mybir.AluOpType.add)

        def scores(b, c, ets):
            # S^T[k, q] for chunk c of batch b; exp tiles into `ets`
            tok0 = b * S
            nk = 4 * c + 4
            q0 = tok0 + c * 512
            for ki in range(nk):
                off = max(0, (ki - 4 * c)) * 128
                w = 512 - off
                ps_s = pss_pool.tile([128, 2, 512], F32, tag="pss")
                for h in range(2):
                    hp = h * 64
                    kslice = qkT_sb[hp:hp + 64, 1,
                                    tok0 + ki * 128: tok0 + (ki + 1) * 128]
                    if ki >= 4 * c:
                        # diagonal tile: scores on first 128 cols, then the
                        # causal mask accumulated on PE, then the rest.
                        nc.tensor.matmul(
                            ps_s[:, h, 0:128], kslice,
                            qkT_sb[hp:hp + 64, 0, q0 + off: q0 + off + 128],
                            start=True, stop=False)
                        nc.tensor.matmul(
                            ps_s[:, h, 0:128], ident_sb, maskT_sb,
                            start=False, stop=True)
                        if w > 128:
                            nc.tensor.matmul(
                                ps_s[:, h, 128:w], kslice,
                                qkT_sb[hp:hp + 64, 0, q0 + off + 128: q0 + 512],
                                start=True, stop=True)
                    else:
                        nc.tensor.matmul(
                            ps_s[:, h, :w], kslice,
                            qkT_sb[hp:hp + 64, 0, q0 + off: q0 + 512],
                            start=True, stop=True)
                et = expp_pool.tile([128, 2, 512], BF16, tag="expp")
                nc.scalar.activation(
                    et[:, :, :w], ps_s[:, :, :w], AF.Exp, scale=SCALE)
                ets[ki] = (et, off)

        def av(b, c, ets):
            # natural AV per (qtile, head): et stationary, [v|1] moving
            for qq in range(4):
                nk = 4 * c + qq + 1
                ps_a = psv_pool.tile([128, 2, 65], F32, tag="psv")
                for h in range(2):
                    for ki in range(nk):
                        et, off = ets[ki]
                        nc.tensor.matmul(
                            ps_a[:, h, :],
                            et[:, h, qq * 128 - off: qq * 128 - off + 128],
                            v_aug[:, b * 16 + ki, h, :],
                            start=(ki == 0), stop=(ki == nk - 1),
                        )
                recip = smalls_pool.tile([128, 2], F32, tag="recip")
                nc.vector.reciprocal(recip[:, :], ps_a[:, :, 64])
                tindex = b * 16 + c * 4 + qq
                for h in range(2):
                    nc.vector.tensor_scalar_mul(
                        av_sb[:, tindex, h * 64:(h + 1) * 64],
                        ps_a[:, h, 0:64], recip[:, h:h + 1])

        def a2a(key, t0, nt):
            # all-to-all over av token-tiles [t0, t0+nt): nt*16-token blocks
            nc.sync.dma_start(
                av_bounce[key].rearrange("(t p) d -> p t d", p=128),
                av_sb[:, t0:t0 + nt, :])
            if _NO_COLLECTIVE:
                nc.sync.dma_start(recv_bounce[key][:, :], av_bounce[key][:, :])
            else:
                nc.gpsimd.collective_compute(
                    "AllToAll", mybir.AluOpType.bypass,
                    replica_groups=[list(range(NCORES))],
                    ins=[av_bounce[key][:, :].opt()],
                    outs=[recv_bounce[key][:, :].opt()],
                )

        def recv_stage(key):
            # DMA-transpose DRAM bounce [(s t), d] -> aT[:, s, t]
            nc.sync.dma_start(
                aT_of[key], recv_bounce[key][:, :], transpose=True)

        def proj(r0, a_slice):
            # out rows r0:r0+128 from a^T slices {s: [128, .., 128-token] AP}
            for n2 in range(2):
                ps = qkvp_pool.tile([128, 512], F32, tag="qkvp")
                for s in range(8):
                    nc.tensor.matmul(
                        ps[:, :],
                        a_slice(s),
                        wp_sb[:, s, n2 * 512:(n2 + 1) * 512],
                        start=(s == 0), stop=(s == 7),
                    )
                o_sb = outs_pool.tile([128, 512], F32, tag="outs")
                nc.scalar.activation(o_sb[:, :], ps[:, :], AF.Copy)
                nc.scalar.dma_start(
                    out[r0:r0 + 128, n2 * 512:(n2 + 1) * 512],
                    o_sb[:, :])

        # ---- fused pipeline. Batch 0: per-chunk QKV/scores with AV lagging
        # one chunk so exp hides behind PE work. Batch 1: QKV bunched, then
        # chunks in DESCENDING order (c3..c0) so the last a2a piece and its
        # exp are the smallest; the last two projections are emitted after
        # the final a2a so PE stays busy through the collective chain.
        ets = {}
        for n in range(4):
            qkv_qk(n)
            vnat(n)
            ets[n] = {}
            scores(0, n, ets[n])
            if n >= 1:
                av(0, n - 1, ets.pop(n - 1))
        qkv_qk(4); vnat(4)
        qkv_qk(5); vnat(5)
        av(0, 3, ets.pop(3))
        a2a(0, 0, 16)
        recv_stage(0)
        qkv_qk(6); vnat(6)
        ets[2] = {}
        scores(1, 2, ets[2])
        qkv_qk(7); vnat(7)
        ets[3] = {}
        scores(1, 3, ets[3])
        av(1, 2, ets.pop(2))
        ets[1] = {}
        scores(1, 1, ets[1])
        av(1, 3, ets.pop(3))
        a2a("1a", 24, 8)
        recv_stage("1a")
        ets[0] = {}
        scores(1, 0, ets[0])
        proj(0, lambda s: aT0[:, s, 0:128])
        av(1, 1, ets.pop(1))
        av(1, 0, ets.pop(0))
        a2a("1b", 16, 8)
        recv_stage("1b")
        proj(128, lambda s: aT0[:, s, 128:256])
        proj(256, lambda s: aT1a[:, s, :])
        proj(384, lambda s: aT1b[:, s, :])

        if dbg is not None:
            nc.sync.dma_start(
                dbg["dbg_qkT"].rearrange("p m n -> p (m n)"),
                qkT_sb[:, :, :].rearrange("p m n -> p (m n)"))
            nc.sync.dma_start(
                dbg["dbg_vaug"].rearrange("p t d -> p (t d)"),
                v_aug[:, :, :, :].rearrange("p t h d -> p (t h d)"))
            nc.sync.dma_start(
                dbg["dbg_av"].rearrange("p t d -> p (t d)"),
                av_sb[:, :, :].rearrange("p t d -> p (t d)"))
            nc.sync.dma_start(
                dbg["dbg_aT"][:, :, 0:256].rearrange("p s n -> p (s n)"),
                aT0[:, :, :].rearrange("p s n -> p (s n)"))
            nc.sync.dma_start(
                dbg["dbg_aT"][:, :, 256:384].rearrange("p s n -> p (s n)"),
                aT1a[:, :, :].rearrange("p s n -> p (s n)"))
            nc.sync.dma_start(
                dbg["dbg_aT"][:, :, 384:512].rearrange("p s n -> p (s n)"),
                aT1b[:, :, :].rearrange("p s n -> p (s n)"))


def _prep_inputs(hidden_states, c_attn_w, c_attn_b, c_proj_w):
    bf16 = ml_dtypes.bfloat16
    x = np.asarray(hidden_states, dtype=np.float32).reshape(NT, D)
    xT = np.ascontiguousarray(x.T).astype(bf16)
    wp = np.ascontiguousarray(np.asarray(c_proj_w, dtype=np.float32)).astype(bf16)
    identity = np.eye(128, dtype=np.float32)
    # maskT[p, f]: S^T diagonal tile entry (k=p, q=f) masked iff q < k
    p = np.arange(128)
    maskT = np.where(p[None, :] >= p[:, None], 0.0, NEG).astype(np.float32)
    cbf16 = np.ascontiguousarray(
        np.concatenate([identity, maskT], axis=1)).astype(bf16)

    w = np.asarray(c_attn_w, dtype=np.float32)
    bb = np.asarray(c_attn_b, dtype=np.float32)
    in_maps = []
    for i in range(NCORES):
        cols = np.r_[i * 128:(i + 1) * 128]
        # pre-swizzled [p(contraction), m, kt, col] -> [128, 3*8*128]
        wshard = np.stack(
            [w[:, cols], w[:, D + cols], w[:, 2 * D + cols]], axis=0
        ).reshape(3, 8, 128, 128).transpose(2, 0, 1, 3).reshape(128, 3072)
        bshard = np.stack(
            [bb[cols], bb[D + cols], bb[2 * D + cols]], axis=1)  # [128, 3]
        bv_bcast = np.tile(bb[2 * D + cols][None, :], (128, 1))  # [128, 128]
        cf32 = np.ascontiguousarray(
            np.concatenate([bshard, bv_bcast], axis=1)).astype(np.float32)
        in_maps.append({
            "xT": xT,
            "wqkv": np.ascontiguousarray(wshard).astype(bf16),
            "wp": wp,
            "cbf16": cbf16,
            "cf32": cf32,
        })
    return in_maps


def kernel(hidden_states, c_attn_w, c_attn_b, c_proj_w, c_proj_b, _trace=False):
    if "nc" not in _CACHE:
        _CACHE["nc"] = _build()
    nc = _CACHE["nc"]
    in_maps = _prep_inputs(hidden_states, c_attn_w, c_attn_b, c_proj_w)
    try:
        res = run_bass_kernel_spmd(nc, in_maps, core_ids=list(range(NCORES)),
                                   trace=_trace)
    except (ImportError, ModuleNotFoundError):
        # NTFF profiling hook unavailable in this container
        res = run_bass_kernel_spmd(nc, in_maps, core_ids=list(range(NCORES)),
                                   trace=False)
    _CACHE["last_result"] = res
    # core j's output rows: [0:256] = batch0 tokens 256j..;
    # [256:384] = batch1 tokens 1024+128j..; [384:512] = batch1 tokens 128j..
    full = np.empty((NT, D), dtype=np.float32)
    for j in range(NCORES):
        o = res.results[j]["out"]
        full[256 * j:256 * (j + 1)] = o[0:256]
        full[S + 1024 + 128 * j:S + 1024 + 128 * (j + 1)] = o[256:384]
        full[S + 128 * j:S + 128 * (j + 1)] = o[384:512]
    full = full + np.asarray(c_proj_b, dtype=np.float32)[None, :]
    return full.reshape(B, S, D).astype(np.float32)
